# revision 1
# baseline (speedup 1.0000x reference)
"""Trainium2 Bass kernel for a two-branch cross-attention block.

Math (per branch pair):
    x1n = LN(x1); x2n = LN(x2)
    q1,k1,v1 = split(x1n @ w_qkv1); q2,k2,v2 = split(x2n @ w_qkv2)
    out1 = softmax(q1 k2^T * s) v2 @ w_out1 + b_out1
    out2 = softmax(q2 k1^T * s) v1 @ w_out2 + b_out2

Sharding: 8 cores = 4 batches x 2 head-groups (8 heads each). Each core
handles both branches for its (batch, head-group); the out-projection
contracts over heads, so each core produces a partial [2048, 1024] per
branch and the host sums the two head-group partials + bias.

LN affine (g, b) is folded into the QKV weights on the host
(W' = diag(g) W, bias' = b W), so the device only computes the pure
normalization z = (x - mu) * rstd. The softmax scale is folded into the
q-side weights. QKV biases enter via K=1 ones-row matmuls.

The two output branches run sequentially on-device, recomputing LN per
use, so the persistent q/k/v/attn tiles fit SBUF (tag-shared slots).

On-device dataflow per output branch (matmul inputs bf16, fp32 accum):
    LN (DVE/ACT) -> PE transpose -> xnT [feat, tok]
    qT, kT via W as stationary; v natural via xnT as stationary
    per head: S^T[j,i] = kT_h^T q_h (K=64) -> exp (ACT, PSUM->SBUF bf16)
              AV: out[65, i] += v_aug[j,:65]^T @ expS^T (col 64 = ones -> Z)
              recipZ = exp(-ln(Z)); DMA partition-broadcast; scale on DVE
    out-proj: attn_T as stationary, accumulate 4 hd-chunks.
"""

import sys
from contextlib import ExitStack

import numpy as np
import ml_dtypes

sys.path.insert(0, "/opt/trn_rl_repo")
sys.path.insert(0, "/opt/trn_rl_repo/concourse")

import concourse.bass as bass
import concourse.tile as tile
from concourse import bacc, mybir
from concourse.bass import ds, ts
from concourse.masks import make_identity

F32 = mybir.dt.float32
BF16 = mybir.dt.bfloat16
AF = mybir.ActivationFunctionType
ALU = mybir.AluOpType

B, N, DIM = 4, 2048, 1024
HEADS, DH = 16, 64
SCALE = DH ** -0.5
HPC = 8          # heads per core
QKCOLS = HPC * DH  # 512 qkv columns per core per tensor
TC = N // 128    # 16 token chunks
KC = DIM // 128  # 8 feature chunks
EPS = 1e-5


def build_program():
    nc = bacc.Bacc(
        "TRN2",
        target_bir_lowering=False,
        debug=False,
        enable_asserts=True,
        num_devices=8,
    )
    xs, wq, wk, wv, bq, bk, bv, wo, outs = [], [], [], [], [], [], [], [], []
    for br in range(2):
        xs.append(nc.dram_tensor(f"x{br}", [N, DIM], F32, kind="ExternalInput").ap())
        wq.append(nc.dram_tensor(f"wq{br}", [DIM, QKCOLS], BF16, kind="ExternalInput").ap())
        wk.append(nc.dram_tensor(f"wk{br}", [DIM, QKCOLS], BF16, kind="ExternalInput").ap())
        wv.append(nc.dram_tensor(f"wv{br}", [DIM, QKCOLS], BF16, kind="ExternalInput").ap())
        bq.append(nc.dram_tensor(f"bq{br}", [1, QKCOLS], BF16, kind="ExternalInput").ap())
        bk.append(nc.dram_tensor(f"bk{br}", [1, QKCOLS], BF16, kind="ExternalInput").ap())
        bv.append(nc.dram_tensor(f"bv{br}", [1, QKCOLS], BF16, kind="ExternalInput").ap())
        wo.append(nc.dram_tensor(f"wo{br}", [QKCOLS, DIM], BF16, kind="ExternalInput").ap())
        outs.append(nc.dram_tensor(f"o{br}", [N, DIM], F32, kind="ExternalOutput").ap())
    # DRAM staging for the per-head recipZ rows (DMA partition-broadcast
    # needs a DRAM source); one row per (ob, h) so there is no reuse.
    zst = nc.dram_tensor("zstage", [2 * HPC, N], F32, kind="Internal").ap()

    with tile.TileContext(nc) as tc:
        with ExitStack() as ctx:
            _body(ctx, tc, xs, wq, wk, wv, bq, bk, bv, wo, outs, zst)
    nc.finalize()
    return nc


def _body(ctx, tc, xs, wq, wk, wv, bq, bk, bv, wo, outs, zst):
    nc = tc.nc
    p_const = ctx.enter_context(tc.tile_pool(name="const", bufs=1))
    p_x = ctx.enter_context(tc.tile_pool(name="x", bufs=3))
    p_stat = ctx.enter_context(tc.tile_pool(name="stat", bufs=4))
    p_z = ctx.enter_context(tc.tile_pool(name="z", bufs=3))
    p_xnT = ctx.enter_context(tc.tile_pool(name="xnT", bufs=1))
    p_w = ctx.enter_context(tc.tile_pool(name="w", bufs=1))
    p_big = ctx.enter_context(tc.tile_pool(name="big", bufs=1))
    p_es = ctx.enter_context(tc.tile_pool(name="es", bufs=3))
    p_rz = ctx.enter_context(tc.tile_pool(name="rz", bufs=1))
    p_outst = ctx.enter_context(tc.tile_pool(name="outst", bufs=2))
    ps_mm = ctx.enter_context(tc.tile_pool(name="ps_mm", bufs=2, space="PSUM"))
    ps_av = ctx.enter_context(tc.tile_pool(name="ps_av", bufs=1, space="PSUM"))

    ident = p_const.tile([128, 128], BF16, tag="ident", name="ident")
    make_identity(nc, ident)
    ones = p_const.tile([1, 512], BF16, tag="ones", name="ones")
    nc.vector.memset(ones, 1.0)
    epst = p_const.tile([128, 1], F32, tag="eps", name="epst")
    nc.vector.memset(epst, EPS)

    def phase_A(br, seg):
        """LN + transpose -> xnT [128, kc, tokens] (bf16).

        Two passes over x (re-DMA'd) so the 16 per-tile Ln/Exp rstd calls
        batch into ONE Ln + ONE Exp -- ACT table sets reload on every
        Ln<->Exp alternation (~2.7us each), which dominated ScalarE time.
        """
        xnT = p_xnT.tile([128, KC, N], BF16, tag="xnT", name=f"xnT_{seg}")
        stats = p_stat.tile([128, TC, 2], F32, tag="stats", name=f"stats_{seg}")
        rstd = p_stat.tile([128, TC], F32, tag="rstd", name=f"rstd_{seg}")
        for t in range(TC):
            xt = p_x.tile([128, DIM], F32, tag="xt", name=f"xt{seg}_{t}")
            nc.sync.dma_start(out=xt, in_=xs[br][ts(t, 128), :])
            st = p_stat.tile([128, 2, 6], F32, tag="st", name=f"st{seg}_{t}")
            for sg in range(2):
                nc.vector.bn_stats(out=st[:, sg, :], in_=xt[:, ts(sg, 512)])
            nc.vector.bn_aggr(out=stats[:, t, :], in_=st)
        # rstd = exp(-0.5 * ln(var + eps)), batched over all 16 tiles
        nc.scalar.activation(out=rstd, in_=stats[:, :, 1], func=AF.Ln,
                             bias=epst, scale=1.0)
        nc.scalar.activation(out=rstd, in_=rstd, func=AF.Exp, scale=-0.5)
        for t in range(TC):
            xt = p_x.tile([128, DIM], F32, tag="xt", name=f"xt2{seg}_{t}")
            nc.sync.dma_start(out=xt, in_=xs[br][ts(t, 128), :])
            zt = p_z.tile([128, DIM], BF16, tag="zt", name=f"zt{seg}_{t}")
            nc.vector.tensor_scalar(out=zt, in0=xt, scalar1=stats[:, t, 0:1],
                                    scalar2=rstd[:, t:t + 1],
                                    op0=ALU.subtract, op1=ALU.mult)
            ptr = ps_mm.tile([128, KC, 128], BF16, tag="mm", name=f"ptr{seg}_{t}")
            for fc in range(KC):
                nc.tensor.transpose(out=ptr[:, fc, :], in_=zt[:, ts(fc, 128)],
                                    identity=ident)
            nc.vector.tensor_copy(out=xnT[:, :, ts(t, 128)], in_=ptr)
        return xnT

    def phase_B(xnT, wt_d, bias_d, dstT, lbl):
        """q or k projection, transposed output layout."""
        w_re = wt_d.rearrange("(kc p) c -> p kc c", p=128)
        for cc in range(4):
            wsb = p_w.tile([128, KC, 128], BF16, tag="w", bufs=2,
                           name=f"w_{lbl}_{cc}")
            nc.sync.dma_start(out=wsb, in_=w_re[:, :, ts(cc, 128)])
            bsb = p_w.tile([1, 128], BF16, tag="b", bufs=2, name=f"b_{lbl}_{cc}")
            nc.sync.dma_start(out=bsb, in_=bias_d[:, ts(cc, 128)])
            for ih in range(2):
                ps = ps_mm.tile([128, 1024], F32, tag="mm",
                                name=f"psB_{lbl}_{cc}_{ih}")
                for i2 in range(2):
                    nc.tensor.matmul(out=ps[:, ts(i2, 512)], lhsT=bsb,
                                     rhs=ones, start=True, stop=False)
                    for k in range(KC):
                        nc.tensor.matmul(
                            out=ps[:, ts(i2, 512)], lhsT=wsb[:, k, :],
                            rhs=xnT[:, k, ds(ih * 1024 + i2 * 512, 512)],
                            start=False, stop=(k == KC - 1))
                nc.vector.tensor_copy(out=dstT[:, cc, ds(ih * 1024, 1024)],
                                      in_=ps)

    def phase_C(xnT, br, vA, seg):
        """v in natural layout [j-part, j-chunk, head, 65] (col 64 = ones)."""
        wvsb = p_w.tile([128, KC, QKCOLS], BF16, tag="wv", name=f"wv_{seg}")
        nc.sync.dma_start(out=wvsb,
                          in_=wv[br].rearrange("(kc p) c -> p kc c", p=128))
        bvsb = p_w.tile([1, QKCOLS], BF16, tag="bv", name=f"bv_{seg}")
        nc.sync.dma_start(out=bvsb, in_=bv[br])
        for j in range(TC):
            ps = ps_mm.tile([128, 1024], F32, tag="mm", name=f"psC_{seg}_{j}")
            nc.tensor.matmul(out=ps[:, 0:512], lhsT=ones[:, 0:128], rhs=bvsb,
                             start=True, stop=False)
            for k in range(KC):
                nc.tensor.matmul(out=ps[:, 0:512], lhsT=xnT[:, k, ts(j, 128)],
                                 rhs=wvsb[:, k, :], start=False,
                                 stop=(k == KC - 1))
            nc.vector.tensor_copy(
                out=vA[:, j, :, 0:DH],
                in_=ps[:, 0:512].rearrange("p (h d) -> p h d", d=DH))

    for ob in range(2):
        sb = 1 - ob
        # prep: q side from branch ob, k/v side from branch sb
        xnT = phase_A(ob, seg=f"{ob}q")
        qT = p_big.tile([128, 4, N], BF16, tag="qT", name=f"qT_{ob}")
        phase_B(xnT, wq[ob], bq[ob], qT, f"q{ob}")
        xnT = phase_A(sb, seg=f"{ob}kv")
        kT = p_big.tile([128, 4, N], BF16, tag="kT", name=f"kT_{ob}")
        phase_B(xnT, wk[sb], bk[sb], kT, f"k{sb}")
        vA = p_big.tile([128, TC, HPC, DH + 1], BF16, tag="vA", name=f"vA_{ob}")
        nc.vector.memset(vA[:, :, :, DH:DH + 1], 1.0)
        phase_C(xnT, sb, vA, seg=f"{ob}")
        aT = p_big.tile([128, 4, N], BF16, tag="aT", name=f"aT_{ob}")

        # ---- attention, one head at a time ----
        # aT receives the UNNORMALIZED output; the per-head Z rows batch
        # into one Ln + one Exp per segment (ACT table sets reload on every
        # Ln<->Exp alternation), then scaling happens in-place on aT.
        zall = p_rz.tile([HPC, N], F32, tag="zall", name=f"zall_{ob}")
        for h in range(HPC):
            pt, po = h // 2, (h % 2) * 64
            avp = ps_av.tile([DH + 1, N], F32, tag="av", name=f"av_{ob}_{h}")
            for j in range(TC):
                es = p_es.tile([128, N], BF16, tag="es", name=f"es_{ob}_{h}_{j}")
                for ih in range(2):
                    ps = ps_mm.tile([128, 1024], F32, tag="mm",
                                    name=f"psS_{ob}_{h}_{j}_{ih}")
                    for i2 in range(2):
                        nc.tensor.matmul(
                            out=ps[:, ts(i2, 512)],
                            lhsT=kT[po:po + 64, pt, ts(j, 128)],
                            rhs=qT[po:po + 64, pt,
                                   ds(ih * 1024 + i2 * 512, 512)],
                            start=True, stop=True)
                    nc.scalar.activation(out=es[:, ts(ih, 1024)], in_=ps,
                                         func=AF.Exp)
                for ib in range(4):
                    nc.tensor.matmul(out=avp[:, ts(ib, 512)],
                                     lhsT=vA[:, j, h, :],
                                     rhs=es[:, ts(ib, 512)],
                                     start=(j == 0), stop=(j == TC - 1))
            # stage unnormalized head output into aT; Z row -> zall[h]
            if po == 0:
                nc.vector.tensor_copy(out=aT[0:64, pt, :], in_=avp[0:64, :])
            else:
                stg = p_rz.tile([64, N], BF16, tag="stg", bufs=2,
                                name=f"stg_{ob}_{h}")
                nc.vector.tensor_copy(out=stg, in_=avp[0:64, :])
                nc.sync.dma_start(out=aT[64:128, pt, :], in_=stg)
            zsb = p_rz.tile([DH + 1, N], F32, tag="zsb", bufs=2,
                            name=f"zsb_{ob}_{h}")
            nc.vector.tensor_copy(out=zsb[64:65, :], in_=avp[64:65, :])
            nc.sync.dma_start(out=zall[h:h + 1, :], in_=zsb[64:65, :])
        # batched recipZ = exp(-ln(Z)) for all 8 heads
        nc.scalar.activation(out=zall, in_=zall, func=AF.Ln)
        nc.scalar.activation(out=zall, in_=zall, func=AF.Exp, scale=-1.0)
        zblk = zst[ob * HPC:(ob + 1) * HPC, :]
        nc.sync.dma_start(out=zblk, in_=zall)
        for h in range(HPC):
            pt, po = h // 2, (h % 2) * 64
            rzb = p_rz.tile([128, N], F32, tag="rzb", bufs=2,
                            name=f"rzb_{ob}_{h}")
            nc.sync.dma_start(
                out=rzb[po:po + 64, :],
                in_=zblk[h:h + 1, :].partition_broadcast(64))
            nc.vector.tensor_mul(out=aT[po:po + 64, pt, :],
                                 in0=aT[po:po + 64, pt, :],
                                 in1=rzb[po:po + 64, :])

        # ---- out-projection (partial over this core's heads) ----
        wosb = p_w.tile([128, 4, DIM], BF16, tag="wo", name=f"wo_{ob}")
        nc.sync.dma_start(out=wosb,
                          in_=wo[ob].rearrange("(hd p) c -> p hd c", p=128))
        for t in range(TC):
            ps = ps_mm.tile([128, 1024], F32, tag="mm", name=f"psE_{ob}_{t}")
            for hd in range(4):
                for cb in range(2):
                    nc.tensor.matmul(out=ps[:, ts(cb, 512)],
                                     lhsT=aT[:, hd, ts(t, 128)],
                                     rhs=wosb[:, hd, ts(cb, 512)],
                                     start=(hd == 0), stop=(hd == 3))
            ot = p_outst.tile([128, DIM], F32, tag="ot", name=f"ot_{ob}_{t}")
            nc.vector.tensor_copy(out=ot, in_=ps)
            nc.sync.dma_start(out=outs[ob][ts(t, 128), :], in_=ot)


_NC = None


def _get_nc():
    global _NC
    if _NC is None:
        _NC = build_program()
    return _NC


def _make_in_maps(x1, x2, ln1_g, ln1_b, ln2_g, ln2_b,
                  w_qkv1, w_qkv2, w_out1, w_out2):
    bf16 = ml_dtypes.bfloat16
    f32 = np.float32
    branches = ((w_qkv1, ln1_g, ln1_b, w_out1), (w_qkv2, ln2_g, ln2_b, w_out2))
    # per head-group g: fold LN affine + softmax scale into weights
    per_g = []
    for g in range(2):
        cols = slice(g * QKCOLS, (g + 1) * QKCOLS)
        m = {}
        for br, (w_qkv, g_ln, b_ln, w_out) in enumerate(branches):
            wq_s = w_qkv[:, 0:DIM][:, cols]
            wk_s = w_qkv[:, DIM:2 * DIM][:, cols]
            wv_s = w_qkv[:, 2 * DIM:3 * DIM][:, cols]
            m[f"wq{br}"] = np.ascontiguousarray(
                (wq_s * g_ln[:, None] * SCALE)).astype(bf16)
            m[f"wk{br}"] = np.ascontiguousarray(wk_s * g_ln[:, None]).astype(bf16)
            m[f"wv{br}"] = np.ascontiguousarray(wv_s * g_ln[:, None]).astype(bf16)
            m[f"bq{br}"] = ((b_ln @ wq_s) * SCALE)[None, :].astype(bf16)
            m[f"bk{br}"] = (b_ln @ wk_s)[None, :].astype(bf16)
            m[f"bv{br}"] = (b_ln @ wv_s)[None, :].astype(bf16)
            m[f"wo{br}"] = np.ascontiguousarray(w_out[cols, :]).astype(bf16)
        per_g.append(m)
    in_maps = []
    for b in range(B):
        for g in range(2):
            m = dict(per_g[g])
            m["x0"] = np.ascontiguousarray(x1[b], dtype=f32)
            m["x1"] = np.ascontiguousarray(x2[b], dtype=f32)
            in_maps.append(m)
    return in_maps


def run(inputs, trace=False):
    """inputs: dict as from setup_inputs(). Returns ((out1, out2), exec_time_ns)."""
    from concourse.bass_utils import run_bass_kernel_spmd

    f32 = np.float32
    ins = {k: np.asarray(v) for k, v in inputs.items()}
    nc = _get_nc()
    in_maps = _make_in_maps(
        ins["x1"].astype(f32), ins["x2"].astype(f32),
        ins["ln1_g"].astype(f32), ins["ln1_b"].astype(f32),
        ins["ln2_g"].astype(f32), ins["ln2_b"].astype(f32),
        ins["w_qkv1"].astype(f32), ins["w_qkv2"].astype(f32),
        ins["w_out1"].astype(f32), ins["w_out2"].astype(f32))
    res = run_bass_kernel_spmd(nc, in_maps, core_ids=list(range(8)), trace=trace)
    r = res.results
    out1 = np.zeros((B, N, DIM), f32)
    out2 = np.zeros((B, N, DIM), f32)
    for b in range(B):
        out1[b] = r[2 * b]["o0"] + r[2 * b + 1]["o0"] + ins["b_out1"].astype(f32)
        out2[b] = r[2 * b]["o1"] + r[2 * b + 1]["o1"] + ins["b_out2"].astype(f32)
    return (out1, out2), res.exec_time_ns


def kernel(**inputs):
    (out1, out2), _ = run(inputs, trace=False)
    return out1, out2

